# revision 1
# baseline (speedup 1.0000x reference)
"""Distributed attention kernel for trn2 (8 NeuronCores).

Problem: B=16, S=4096, D=64 attention, out = softmax(Q K^T / sqrt(D)) V.
Sharding: batch dim B across 8 cores (2 batches per core), no collectives.

Per-core algorithm (all in "transposed score" layout to avoid on-chip
transposes of the big S x S score matrix):
  - Load Q, K naturally, transpose [128,64] chunks via TensorE into
    QT/KT [64, 4096] (d on partitions).
  - Load V into V_aug [128, 65] chunks with a ones column appended.
  - For each 512-wide q tile:
      for each k chunk of 128: S^T[k,q] = KT_chunk.T @ QT_tile (PSUM),
      exp via ScalarE (scale=1/8) PSUM->SBUF,
      outT[d(+sum),q] += V_aug_chunk.T @ expT  (PSUM accumulate).
    Then transpose outT back 128 q at a time, divide by the sum row,
    DMA out contiguously.
"""

import numpy as np

import concourse.bass as bass
import concourse.mybir as mybir
from concourse import bacc
from concourse.tile import TileContext
from concourse.bass_utils import run_bass_kernel_spmd
from concourse.masks import make_identity

B, S, D = 16, 4096, 64
N_CORES = 8
BS = B // N_CORES  # batches per core
SCALE = 1.0 / np.sqrt(D)  # 0.125
F32 = mybir.dt.float32

QTW = 512  # q-tile width
KP = 2  # k-chunks per exp group
N_KC = S // 128  # 32 k chunks
N_QT = S // QTW  # 8 q tiles


def build_body(nc, tc, Qd, Kd, Vd, Od):
    with (
        tc.tile_pool(name="const", bufs=1) as constp,
        tc.tile_pool(name="qk", bufs=2) as qkp,
        tc.tile_pool(name="vaug", bufs=2) as vp,
        tc.tile_pool(name="nat", bufs=2) as natp,
        tc.tile_pool(name="spsum", bufs=2, space="PSUM") as spp,
        tc.tile_pool(name="opsum", bufs=2, space="PSUM") as opp,
        tc.tile_pool(name="smallpsum", bufs=2, space="PSUM") as tpp,
        tc.tile_pool(name="expt", bufs=3) as ep,
        tc.tile_pool(name="fin", bufs=4) as fp,
    ):
        ident = constp.tile([128, 128], F32)
        make_identity(nc, ident[:])

        for b in range(BS):
            # ---- Phase A: load and transpose Q, K; load V (+ ones col) ----
            qt = qkp.tile([64, S], F32, tag="qt")
            kt = qkp.tile([64, S], F32, tag="kt")
            vaug = vp.tile([128, 65 * N_KC], F32, tag="vaug")
            qnat = natp.tile([128, N_KC * 64], F32, tag="qnat")
            knat = natp.tile([128, N_KC * 64], F32, tag="knat")

            nc.sync.dma_start(
                out=qnat[:].rearrange("p (c d) -> p c d", d=64),
                in_=Qd[b].rearrange("(c p) d -> p c d", p=128),
            )
            nc.sync.dma_start(
                out=knat[:].rearrange("p (c d) -> p c d", d=64),
                in_=Kd[b].rearrange("(c p) d -> p c d", p=128),
            )
            vaug_3d = vaug[:].rearrange("p (c e) -> p c e", e=65)
            nc.sync.dma_start(
                out=vaug_3d[:, :, 0:64],
                in_=Vd[b].rearrange("(c p) d -> p c d", p=128),
            )
            nc.gpsimd.memset(vaug_3d[:, :, 64:65], 1.0)

            for c in range(N_KC):
                trq = tpp.tile([64, 128], F32, tag="small")
                nc.tensor.transpose(trq[:], qnat[:, c * 64 : (c + 1) * 64], ident[:])
                nc.vector.tensor_copy(qt[:, c * 128 : (c + 1) * 128], trq[:])
                trk = tpp.tile([64, 128], F32, tag="small")
                nc.tensor.transpose(trk[:], knat[:, c * 64 : (c + 1) * 64], ident[:])
                nc.vector.tensor_copy(kt[:, c * 128 : (c + 1) * 128], trk[:])

            # ---- Phase B: streaming softmax(QK^T)V in transposed layout ----
            for qi in range(N_QT):
                qs = qt[:, qi * QTW : (qi + 1) * QTW]
                ot = opp.tile([65, QTW], F32, tag="ot")
                for g in range(N_KC // KP):
                    sp = spp.tile([128, QTW * KP], F32, tag="sp")
                    for j in range(KP):
                        c = g * KP + j
                        nc.tensor.matmul(
                            sp[:, j * QTW : (j + 1) * QTW],
                            kt[:, c * 128 : (c + 1) * 128],
                            qs,
                            start=True,
                            stop=True,
                            skip_group_check=True,
                        )
                    et = ep.tile([128, QTW * KP], F32, tag="et")
                    nc.scalar.activation(
                        et[:], sp[:], mybir.ActivationFunctionType.Exp, scale=SCALE
                    )
                    for j in range(KP):
                        c = g * KP + j
                        nc.tensor.matmul(
                            ot[:],
                            vaug_3d[:, c, :],
                            et[:, j * QTW : (j + 1) * QTW],
                            start=(c == 0),
                            stop=(c == N_KC - 1),
                            skip_group_check=True,
                        )

                # ---- Phase C: transpose back, normalize, store ----
                osb = fp.tile([65, QTW], F32, tag="osb")
                nc.vector.tensor_copy(osb[:], ot[:])
                for j in range(QTW // 128):
                    ftp = tpp.tile([128, 65], F32, tag="small")
                    nc.tensor.transpose(
                        ftp[:], osb[:, j * 128 : (j + 1) * 128], ident[0:65, 0:65]
                    )
                    rinv = fp.tile([128, 1], F32, tag="rinv")
                    nc.vector.reciprocal(rinv[:], ftp[:, 64:65])
                    ob = fp.tile([128, 64], F32, tag="ob")
                    nc.vector.tensor_scalar_mul(ob[:], ftp[:, 0:64], rinv[:])
                    q0 = qi * QTW + j * 128
                    nc.sync.dma_start(out=Od[b, q0 : q0 + 128, :], in_=ob[:])


_nc_cache = None


def build_nc():
    global _nc_cache
    if _nc_cache is not None:
        return _nc_cache
    nc = bacc.Bacc(None, target_bir_lowering=False)
    Qd = nc.declare_dram_parameter("Q", [BS, S, D], F32, isOutput=False)
    Kd = nc.declare_dram_parameter("K", [BS, S, D], F32, isOutput=False)
    Vd = nc.declare_dram_parameter("V", [BS, S, D], F32, isOutput=False)
    Od = nc.declare_dram_parameter("out", [BS, S, D], F32, isOutput=True)
    with TileContext(nc) as tc:
        build_body(nc, tc, Qd, Kd, Vd, Od)
    nc.finalize()
    _nc_cache = nc
    return nc


def kernel(Q, K, V):
    Q = np.asarray(Q, dtype=np.float32)
    K = np.asarray(K, dtype=np.float32)
    V = np.asarray(V, dtype=np.float32)
    nc = build_nc()
    in_maps = [
        {
            "Q": np.ascontiguousarray(Q[i * BS : (i + 1) * BS]),
            "K": np.ascontiguousarray(K[i * BS : (i + 1) * BS]),
            "V": np.ascontiguousarray(V[i * BS : (i + 1) * BS]),
        }
        for i in range(N_CORES)
    ]
    res = run_bass_kernel_spmd(nc, in_maps, core_ids=list(range(N_CORES)))
    return np.concatenate([res.results[i]["out"] for i in range(N_CORES)], axis=0)


# revision 3
# speedup vs baseline: 2.0924x; 2.0924x over previous
"""Distributed attention kernel for trn2 (8 NeuronCores).

Problem: B=16, S=4096, D=64 attention, out = softmax(Q K^T / sqrt(D)) V.
Sharding: batch dim B across 8 cores (2 batches per core), no collectives.

Per-core algorithm (all in "transposed score" layout to avoid on-chip
transposes of the big S x S score matrix):
  - Load Q, K naturally, transpose [128,64] chunks via TensorE into
    QT/KT [64, 4096] (d on partitions).
  - Load V into V_aug [128, 65] chunks with a ones column appended.
  - For each 512-wide q tile:
      for each k chunk of 128: S^T[k,q] = KT_chunk.T @ QT_tile (PSUM),
      exp via ScalarE (scale=1/8) PSUM->SBUF,
      outT[d(+sum),q] += V_aug_chunk.T @ expT  (PSUM accumulate).
    Then transpose outT back 128 q at a time, divide by the sum row,
    DMA out contiguously.
"""

import numpy as np

import concourse.bass as bass
import concourse.mybir as mybir
from concourse import bacc
from concourse.tile import TileContext
from concourse.bass_utils import run_bass_kernel_spmd
from concourse.masks import make_identity

B, S, D = 16, 4096, 64
N_CORES = 8
BS = B // N_CORES  # batches per core
SCALE = 1.0 / np.sqrt(D)  # 0.125
F32 = mybir.dt.float32
BF16 = mybir.dt.bfloat16

QTW = 512  # q-tile width
KP = 2  # k-chunks per exp group
N_KC = S // 128  # 32 k chunks
N_QT = S // QTW  # 8 q tiles


def build_body(nc, tc, Qd, Kd, Vd, Od):
    with (
        tc.tile_pool(name="const", bufs=1) as constp,
        tc.tile_pool(name="qk", bufs=2) as qkp,
        tc.tile_pool(name="vaug", bufs=2) as vp,
        tc.tile_pool(name="nat", bufs=2) as natp,
        tc.tile_pool(name="spsum", bufs=2, space="PSUM") as spp,
        tc.tile_pool(name="opsum", bufs=2, space="PSUM") as opp,
        tc.tile_pool(name="smallpsum", bufs=2, space="PSUM") as tpp,
        tc.tile_pool(name="expt", bufs=3) as ep,
        tc.tile_pool(name="fin", bufs=4) as fp,
    ):
        ident = constp.tile([128, 128], F32)
        make_identity(nc, ident[:])

        for b in range(BS):
            # ---- Phase A: load and transpose Q, K; load V (+ ones col) ----
            qt = qkp.tile([64, S], BF16, tag="qt")
            kt = qkp.tile([64, S], BF16, tag="kt")
            vaug = vp.tile([128, 65 * N_KC], BF16, tag="vaug")
            qnat = natp.tile([128, N_KC * 64], F32, tag="qnat")
            knat = natp.tile([128, N_KC * 64], F32, tag="knat")

            nc.sync.dma_start(
                out=qnat[:].rearrange("p (c d) -> p c d", d=64),
                in_=Qd[b].rearrange("(c p) d -> p c d", p=128),
            )
            nc.sync.dma_start(
                out=knat[:].rearrange("p (c d) -> p c d", d=64),
                in_=Kd[b].rearrange("(c p) d -> p c d", p=128),
            )
            vaug_3d = vaug[:].rearrange("p (c e) -> p c e", e=65)
            nc.gpsimd.dma_start(
                out=vaug_3d[:, :, 0:64],
                in_=Vd[b].rearrange("(c p) d -> p c d", p=128),
            )
            nc.gpsimd.memset(vaug_3d[:, :, 64:65], 1.0)

            for c in range(N_KC):
                trq = tpp.tile([64, 128], F32, tag="small")
                nc.tensor.transpose(trq[:], qnat[:, c * 64 : (c + 1) * 64], ident[:])
                nc.vector.tensor_copy(qt[:, c * 128 : (c + 1) * 128], trq[:])
                trk = tpp.tile([64, 128], F32, tag="small")
                nc.tensor.transpose(trk[:], knat[:, c * 64 : (c + 1) * 64], ident[:])
                nc.vector.tensor_copy(kt[:, c * 128 : (c + 1) * 128], trk[:])

            # ---- Phase B: streaming softmax(QK^T)V in transposed layout ----
            for qi in range(N_QT):
                qs = qt[:, qi * QTW : (qi + 1) * QTW]
                ot = opp.tile([65, QTW], F32, tag="ot")
                for g in range(N_KC // KP):
                    sp = spp.tile([128, QTW * KP], F32, tag="sp")
                    for j in range(KP):
                        c = g * KP + j
                        nc.tensor.matmul(
                            sp[:, j * QTW : (j + 1) * QTW],
                            kt[:, c * 128 : (c + 1) * 128],
                            qs,
                            start=True,
                            stop=True,
                            skip_group_check=True,
                        )
                    et = ep.tile([128, QTW * KP], BF16, tag="et")
                    nc.scalar.activation(
                        et[:], sp[:], mybir.ActivationFunctionType.Exp, scale=SCALE
                    )
                    for j in range(KP):
                        c = g * KP + j
                        nc.tensor.matmul(
                            ot[:],
                            vaug_3d[:, c, :],
                            et[:, j * QTW : (j + 1) * QTW],
                            start=(c == 0),
                            stop=(c == N_KC - 1),
                            skip_group_check=True,
                        )

                # ---- Phase C: transpose back, normalize, store ----
                osb = fp.tile([65, QTW], F32, tag="osb")
                nc.vector.tensor_copy(osb[:], ot[:])
                for j in range(QTW // 128):
                    ftp = tpp.tile([128, 65], F32, tag="small")
                    nc.tensor.transpose(
                        ftp[:], osb[:, j * 128 : (j + 1) * 128], ident[0:65, 0:65]
                    )
                    rinv = fp.tile([128, 1], F32, tag="rinv")
                    nc.vector.reciprocal(rinv[:], ftp[:, 64:65])
                    ob = fp.tile([128, 64], F32, tag="ob")
                    nc.vector.tensor_scalar_mul(ob[:], ftp[:, 0:64], rinv[:])
                    q0 = qi * QTW + j * 128
                    nc.sync.dma_start(out=Od[b, q0 : q0 + 128, :], in_=ob[:])


_nc_cache = None


def build_nc():
    global _nc_cache
    if _nc_cache is not None:
        return _nc_cache
    nc = bacc.Bacc(None, target_bir_lowering=False)
    Qd = nc.declare_dram_parameter("Q", [BS, S, D], F32, isOutput=False)
    Kd = nc.declare_dram_parameter("K", [BS, S, D], F32, isOutput=False)
    Vd = nc.declare_dram_parameter("V", [BS, S, D], F32, isOutput=False)
    Od = nc.declare_dram_parameter("out", [BS, S, D], F32, isOutput=True)
    with TileContext(nc) as tc:
        build_body(nc, tc, Qd, Kd, Vd, Od)
    nc.finalize()
    _nc_cache = nc
    return nc


def kernel(Q, K, V):
    Q = np.asarray(Q, dtype=np.float32)
    K = np.asarray(K, dtype=np.float32)
    V = np.asarray(V, dtype=np.float32)
    nc = build_nc()
    in_maps = [
        {
            "Q": np.ascontiguousarray(Q[i * BS : (i + 1) * BS]),
            "K": np.ascontiguousarray(K[i * BS : (i + 1) * BS]),
            "V": np.ascontiguousarray(V[i * BS : (i + 1) * BS]),
        }
        for i in range(N_CORES)
    ]
    res = run_bass_kernel_spmd(nc, in_maps, core_ids=list(range(N_CORES)))
    return np.concatenate([res.results[i]["out"] for i in range(N_CORES)], axis=0)


# revision 9
# speedup vs baseline: 2.3093x; 1.1037x over previous
"""Distributed attention kernel for trn2 (8 NeuronCores).

Problem: B=16, S=4096, D=64 attention, out = softmax(Q K^T / sqrt(D)) V.
Sharding: batch dim B across 8 cores (2 batches per core), no collectives.

Per-core dataflow (everything in "transposed score" layout; PE assumed
pinned at 1.2 GHz, so all matmuls are packed with tile_position
concurrency):
  - K^T loaded via bf16 DRAM scratch + x-bar DMA transpose in an
    even/odd-s interleaved layout: ktp[128, 2048], top half = K^T of
    even s, bottom half = odd s.
  - Q^T duplicated onto both partition halves (qt2[128, 4096]) via
    doubled load + TensorE transposes, so 2x2-tiled score matmuls can
    source rhs from either half.
  - V loaded even/odd interleaved (vq[128, g, parity, 64]).
  - Per 512-wide q tile, per group g (256 consecutive k):
      S-quad: 4 concurrent K=64/M=64 matmuls -> S^T for even k (bank 0)
              and odd k (bank 1) of sp[128, 1024].
      exp:    one ScalarE activation [128, 1024] psum -> et bf16.
      AV-quad: 4 concurrent M=32 col-tiled matmuls accumulate
              O_even (ot2[0:64]) and O_odd (ot2[64:128]).
      sums:   every 2nd group, 4 concurrent M=1 matmuls with a ones
              vector accumulate sum(exp) into rows {0,32,64,96} of rs.
  - Phase C: copy to SBUF, accumulating PE transposes add the even/odd
    halves, a tiny matmul with a 4-hot selector vector folds the 4 sum
    rows into r[q], reciprocal + per-partition scale, contiguous DMA.
"""

import numpy as np

import concourse.bass as bass
import concourse.mybir as mybir
from concourse import bacc
from concourse.tile import TileContext
from concourse.bass_utils import run_bass_kernel_spmd
from concourse.masks import make_identity

B, S, D = 16, 4096, 64
N_CORES = 8
BS = B // N_CORES  # batches per core
SCALE = 1.0 / np.sqrt(D)  # 0.125
F32 = mybir.dt.float32
BF16 = mybir.dt.bfloat16

QTW = 512  # q-tile width
NG = S // 256  # 16 groups of 256 k (even/odd chunk pair) per q tile
N_QT = S // QTW  # 8 q tiles
NCH = S // 128  # 32 chunks of 128 rows


def emit_phase_a(nc, pools, Qd, Kd, Vd, b, identB):
    qkp, vp, natp, drp, tpp = (
        pools["qk"], pools["v"], pools["nat"], pools["dram"], pools["small"],
    )
    # K: cast to bf16 scratch, then x-bar transpose-DMA into the
    # even/odd interleaved layout ktp[p, a]: p<64 -> K[2a, p] (even s),
    # p>=64 -> K[2a+1, p-64].
    kscr = drp.tile([2048, 128], BF16, tag="kscr")
    nc.gpsimd.dma_start(
        out=kscr[:].rearrange("a (two d) -> (a two) d", two=2),
        in_=Kd[b],
    )
    ktp = qkp.tile([128, 2048], BF16, tag="ktp")
    nc.sync.dma_start(out=ktp[:], in_=kscr[:], transpose=True)

    # V: even/odd interleaved chunks vq[p, g, parity, d]:
    # row r of chunk (g, par) = V[256g + 2r + par, :]
    vq = vp.tile([128, NG * 2 * 64], BF16, tag="vq")
    vq4 = vq[:].rearrange("p (g par d) -> p g par d", par=2, d=64)
    nc.gpsimd.dma_start(
        out=vq4,
        in_=Vd[b].rearrange("(g r par) d -> r g par d", par=2, r=128),
    )

    # Q: doubled natural load (both 64-col halves hold the same chunk),
    # then TensorE-transpose each [128, 128] block -> [128, 128] with
    # Q^T duplicated on both partition halves.
    qt2 = qkp.tile([128, S], BF16, tag="qt2")
    for piece in range(4):
        c0, c1 = piece * 8, (piece + 1) * 8
        qnat2 = natp.tile([128, 8 * 2 * 64], BF16, tag=f"qnat{piece % 2}")
        qn4 = qnat2[:].rearrange("p (c two d) -> p c two d", two=2, d=64)
        src = Qd[b, c0 * 128 : c1 * 128].rearrange("(c p) d -> p c d", p=128)
        nc.gpsimd.dma_start(out=qn4[:, :, 0, :], in_=src)
        nc.gpsimd.dma_start(out=qn4[:, :, 1, :], in_=src)
        for c in range(c0, c1):
            trq = tpp.tile([128, 128], BF16, tag="small")
            nc.tensor.transpose(
                trq[:], qnat2[:, (c - c0) * 128 : (c - c0 + 1) * 128], identB[:]
            )
            nc.vector.tensor_copy(qt2[:, c * 128 : (c + 1) * 128], trq[:])
    return qt2, ktp, vq4


def emit_phase_b(nc, pools, Od, b, qt2, ktp, vq4, ones, wsel, ident2, after_qt0):
    spp, opp, rsp, tpp, ep, fp = (
        pools["sp"], pools["ot"], pools["rs"], pools["small"],
        pools["et"], pools["fin"],
    )
    NGG = N_QT * NG  # 128 groups per batch
    sp_tiles = {}

    def emit_squad(gg):
        qi, g = gg // NG, gg % NG
        qc_lo = qt2[0:64, qi * QTW : (qi + 1) * QTW]
        qc_hi = qt2[64:128, qi * QTW : (qi + 1) * QTW]
        sp = spp.tile([128, QTW * 2], F32, tag="sp")
        sp_tiles[gg] = sp
        nc.tensor.matmul(
            sp[0:64, 0:QTW], ktp[0:64, g * 128 : g * 128 + 64], qc_lo,
            start=True, stop=True, skip_group_check=True,
        )
        nc.tensor.matmul(
            sp[64:128, 0:QTW], ktp[0:64, g * 128 + 64 : g * 128 + 128], qc_lo,
            start=True, stop=True, skip_group_check=True,
        )
        nc.tensor.matmul(
            sp[0:64, QTW : 2 * QTW], ktp[64:128, g * 128 : g * 128 + 64], qc_hi,
            start=True, stop=True, skip_group_check=True,
        )
        nc.tensor.matmul(
            sp[64:128, QTW : 2 * QTW],
            ktp[64:128, g * 128 + 64 : g * 128 + 128], qc_hi,
            start=True, stop=True, skip_group_check=True,
        )

    emit_squad(0)
    emit_squad(1)
    ot2 = rs = None
    et_prev = None
    for gg in range(NGG):
        qi, g = gg // NG, gg % NG
        if g == 0:
            ot2 = opp.tile([128, QTW], F32, tag="ot2")
            rs = rsp.tile([97, QTW], F32, tag="rs")
        sp = sp_tiles.pop(gg)
        et = ep.tile([128, QTW * 2], BF16, tag="et")
        nc.scalar.activation(
            et[:], sp[:], mybir.ActivationFunctionType.Exp, scale=SCALE
        )
        for t in range(4):
            par = t // 2
            nc.tensor.matmul(
                ot2[32 * t : 32 * (t + 1), :],
                vq4[:, g, par, 32 * (t % 2) : 32 * (t % 2 + 1)],
                et[:, par * QTW : (par + 1) * QTW],
                start=(g == 0), stop=(g == NG - 1), skip_group_check=True,
                tile_position=(0, 32 * t),
            )
        if g % 2 == 1:
            for t, (esrc, half) in enumerate(
                [(et_prev, 0), (et_prev, 1), (et, 0), (et, 1)]
            ):
                nc.tensor.matmul(
                    rs[32 * t : 32 * t + 1, :],
                    ones[:],
                    esrc[:, half * QTW : (half + 1) * QTW],
                    start=(g == 1), stop=(g == NG - 1),
                    skip_group_check=True, tile_position=(0, 32 * t),
                )
        et_prev = et
        if gg + 2 < NGG:
            emit_squad(gg + 2)

        if g == NG - 1:
            # ---- Phase C for q-tile qi ----
            osb = fp.tile([128, QTW], BF16, tag="osb")
            nc.vector.tensor_copy(osb[:], ot2[:])
            rsb = fp.tile([97, QTW], BF16, tag="rsb")
            nc.vector.tensor_copy(rsb[:], rs[:])
            for j in range(QTW // 128):
                js = slice(j * 128, (j + 1) * 128)
                ctp = tpp.tile([128, 64], F32, tag="small")
                nc.tensor.matmul(
                    ctp[:], osb[:, js], ident2[:],
                    start=True, stop=True, skip_group_check=True,
                )
                rcol = rsp.tile([128, 1], F32, tag="rs")
                nc.tensor.matmul(
                    rcol[:], rsb[:, js], wsel[:],
                    start=True, stop=True, skip_group_check=True,
                )
                rinv = fp.tile([128, 1], F32, tag="rinv")
                nc.vector.reciprocal(rinv[:], rcol[:])
                ob = fp.tile([128, 64], F32, tag="ob")
                nc.vector.tensor_scalar_mul(ob[:], ctp[:], rinv[:])
                q0 = qi * QTW + j * 128
                nc.sync.dma_start(out=Od[b, q0 : q0 + 128, :], in_=ob[:])
            if qi == 0 and after_qt0 is not None:
                after_qt0()


def build_body(nc, tc, Qd, Kd, Vd, Od):
    with (
        tc.tile_pool(name="const", bufs=1) as constp,
        tc.tile_pool(name="qk", bufs=2) as qkp,
        tc.tile_pool(name="v", bufs=2) as vp,
        tc.tile_pool(name="nat", bufs=2) as natp,
        tc.tile_pool(name="dram", bufs=2, space="DRAM") as drp,
        tc.tile_pool(name="sp", bufs=2, space="PSUM") as spp,
        tc.tile_pool(name="ot", bufs=2, space="PSUM") as opp,
        tc.tile_pool(name="rs", bufs=1, space="PSUM") as rsp,
        tc.tile_pool(name="small", bufs=1, space="PSUM") as tpp,
        tc.tile_pool(name="et", bufs=3) as ep,
        tc.tile_pool(name="fin", bufs=4) as fp,
    ):
        pools = {
            "qk": qkp, "v": vp, "nat": natp, "dram": drp, "sp": spp,
            "ot": opp, "rs": rsp, "small": tpp, "et": ep, "fin": fp,
        }
        ident2 = constp.tile([128, 64], BF16)
        nc.gpsimd.memset(ident2[:], 0.0)
        for half in range(2):
            nc.gpsimd.affine_select(
                out=ident2[64 * half : 64 * (half + 1), :],
                in_=ident2[64 * half : 64 * (half + 1), :],
                compare_op=mybir.AluOpType.not_equal, fill=1.0, base=0,
                pattern=[[-1, 64]], channel_multiplier=1,
            )
        identB = constp.tile([128, 128], BF16)
        make_identity(nc, identB[:])
        ones = constp.tile([128, 1], BF16)
        nc.gpsimd.memset(ones[:], 1.0)
        wsel = constp.tile([97, 1], BF16)
        nc.gpsimd.memset(wsel[:], 0.0)
        for t in range(4):
            nc.gpsimd.memset(wsel[32 * t : 32 * t + 1, :], 1.0)

        handles = [None] * BS
        handles[0] = emit_phase_a(nc, pools, Qd, Kd, Vd, 0, identB)
        for b in range(BS):

            def prefetch(b=b):
                if b + 1 < BS:
                    handles[b + 1] = emit_phase_a(
                        nc, pools, Qd, Kd, Vd, b + 1, identB
                    )

            qt2, ktp, vq4 = handles[b]
            emit_phase_b(
                nc, pools, Od, b, qt2, ktp, vq4, ones, wsel, ident2, prefetch
            )


_nc_cache = None


def build_nc():
    global _nc_cache
    if _nc_cache is not None:
        return _nc_cache
    nc = bacc.Bacc(None, target_bir_lowering=False)
    Qd = nc.declare_dram_parameter("Q", [BS, S, D], F32, isOutput=False)
    Kd = nc.declare_dram_parameter("K", [BS, S, D], F32, isOutput=False)
    Vd = nc.declare_dram_parameter("V", [BS, S, D], F32, isOutput=False)
    Od = nc.declare_dram_parameter("out", [BS, S, D], F32, isOutput=True)
    with TileContext(nc) as tc:
        build_body(nc, tc, Qd, Kd, Vd, Od)
    nc.finalize()
    _nc_cache = nc
    return nc


def kernel(Q, K, V):
    Q = np.asarray(Q, dtype=np.float32)
    K = np.asarray(K, dtype=np.float32)
    V = np.asarray(V, dtype=np.float32)
    nc = build_nc()
    in_maps = [
        {
            "Q": np.ascontiguousarray(Q[i * BS : (i + 1) * BS]),
            "K": np.ascontiguousarray(K[i * BS : (i + 1) * BS]),
            "V": np.ascontiguousarray(V[i * BS : (i + 1) * BS]),
        }
        for i in range(N_CORES)
    ]
    res = run_bass_kernel_spmd(nc, in_maps, core_ids=list(range(N_CORES)))
    return np.concatenate([res.results[i]["out"] for i in range(N_CORES)], axis=0)


# revision 10
# speedup vs baseline: 2.5797x; 1.1171x over previous
"""Distributed attention kernel for trn2 (8 NeuronCores).

Problem: B=16, S=4096, D=64 attention, out = softmax(Q K^T / sqrt(D)) V.
Sharding: batch dim B across 8 cores (2 batches per core), no collectives.

Per-core dataflow (everything in "transposed score" layout; PE assumed
pinned at 1.2 GHz, so all matmuls are packed with tile_position
concurrency):
  - K^T loaded via bf16 DRAM scratch + x-bar DMA transpose in an
    even/odd-s interleaved layout: ktp[128, 2048], top half = K^T of
    even s, bottom half = odd s.
  - Q^T duplicated onto both partition halves (qt2[128, 4096]) via
    doubled load + TensorE transposes, so 2x2-tiled score matmuls can
    source rhs from either half.
  - V loaded even/odd interleaved (vq[128, g, parity, 64]).
  - Per 512-wide q tile, per group g (256 consecutive k):
      S-quad: 4 concurrent K=64/M=64 matmuls -> S^T for even k (bank 0)
              and odd k (bank 1) of sp[128, 1024].
      exp:    one ScalarE activation [128, 1024] psum -> et bf16.
      AV-quad: 4 concurrent M=32 col-tiled matmuls accumulate
              O_even (ot2[0:64]) and O_odd (ot2[64:128]).
      sums:   every 2nd group, 4 concurrent M=1 matmuls with a ones
              vector accumulate sum(exp) into rows {0,32,64,96} of rs.
  - Phase C: copy to SBUF, accumulating PE transposes add the even/odd
    halves, a tiny matmul with a 4-hot selector vector folds the 4 sum
    rows into r[q], reciprocal + per-partition scale, contiguous DMA.
"""

import numpy as np

import concourse.bass as bass
import concourse.mybir as mybir
from concourse import bacc
from concourse.tile import TileContext
from concourse.bass_utils import run_bass_kernel_spmd
from concourse.masks import make_identity

B, S, D = 16, 4096, 64
N_CORES = 8
BS = B // N_CORES  # batches per core
SCALE = 1.0 / np.sqrt(D)  # 0.125
F32 = mybir.dt.float32
BF16 = mybir.dt.bfloat16

QTW = 512  # q-tile width
NG = S // 256  # 16 groups of 256 k (even/odd chunk pair) per q tile
N_QT = S // QTW  # 8 q tiles
NCH = S // 128  # 32 chunks of 128 rows


def emit_phase_a(nc, pools, Qd, Kd, Vd, b, identB):
    qkp, vp, natp, drp, tpp = (
        pools["qk"], pools["v"], pools["nat"], pools["dram"], pools["small"],
    )
    # K: cast to bf16 scratch, then x-bar transpose-DMA into the
    # even/odd interleaved layout ktp[p, a]: p<64 -> K[2a, p] (even s),
    # p>=64 -> K[2a+1, p-64].
    kscr = drp.tile([2048, 128], BF16, tag="kscr")
    nc.gpsimd.dma_start(
        out=kscr[:].rearrange("a (two d) -> (a two) d", two=2),
        in_=Kd[b],
    )
    ktp = qkp.tile([128, 2048], BF16, tag="ktp")
    nc.sync.dma_start(out=ktp[:], in_=kscr[:], transpose=True)

    # V: even/odd interleaved chunks vq[p, g, parity, d]:
    # row r of chunk (g, par) = V[256g + 2r + par, :]
    vq = vp.tile([128, NG * 2 * 64], BF16, tag="vq")
    vq4 = vq[:].rearrange("p (g par d) -> p g par d", par=2, d=64)
    nc.gpsimd.dma_start(
        out=vq4,
        in_=Vd[b].rearrange("(g r par) d -> r g par d", par=2, r=128),
    )

    # Q: doubled natural load (both 64-col halves hold the same chunk),
    # then TensorE-transpose each [128, 128] block -> [128, 128] with
    # Q^T duplicated on both partition halves.
    qt2 = qkp.tile([128, S], BF16, tag="qt2")
    for piece in range(4):
        c0, c1 = piece * 8, (piece + 1) * 8
        qnat2 = natp.tile([128, 8 * 2 * 64], BF16, tag=f"qnat{piece % 2}")
        qn4 = qnat2[:].rearrange("p (c two d) -> p c two d", two=2, d=64)
        src = Qd[b, c0 * 128 : c1 * 128].rearrange("(c p) d -> p c d", p=128)
        nc.gpsimd.dma_start(out=qn4[:, :, 0, :], in_=src)
        nc.gpsimd.dma_start(out=qn4[:, :, 1, :], in_=src)
        for c in range(c0, c1):
            trq = tpp.tile([128, 128], BF16, tag="small")
            nc.tensor.transpose(
                trq[:], qnat2[:, (c - c0) * 128 : (c - c0 + 1) * 128], identB[:]
            )
            nc.vector.tensor_copy(qt2[:, c * 128 : (c + 1) * 128], trq[:])
    return qt2, ktp, vq4


def emit_phase_b(nc, pools, Od, b, qt2, ktp, vq4, ones, wsel, ident2, after_qt0):
    spp, opp, rsp, tpp, ep, fp = (
        pools["sp"], pools["ot"], pools["rs"], pools["small"],
        pools["et"], pools["fin"],
    )
    NGG = N_QT * NG  # 128 groups per batch
    sp_tiles = {}

    def emit_squad(gg):
        qi, g = gg // NG, gg % NG
        qc_lo = qt2[0:64, qi * QTW : (qi + 1) * QTW]
        qc_hi = qt2[64:128, qi * QTW : (qi + 1) * QTW]
        sp = spp.tile([128, QTW * 2], F32, tag="sp")
        sp_tiles[gg] = sp
        nc.tensor.matmul(
            sp[0:64, 0:QTW], ktp[0:64, g * 128 : g * 128 + 64], qc_lo,
            start=True, stop=True, skip_group_check=True,
        )
        nc.tensor.matmul(
            sp[64:128, 0:QTW], ktp[0:64, g * 128 + 64 : g * 128 + 128], qc_lo,
            start=True, stop=True, skip_group_check=True,
        )
        nc.tensor.matmul(
            sp[0:64, QTW : 2 * QTW], ktp[64:128, g * 128 : g * 128 + 64], qc_hi,
            start=True, stop=True, skip_group_check=True,
        )
        nc.tensor.matmul(
            sp[64:128, QTW : 2 * QTW],
            ktp[64:128, g * 128 + 64 : g * 128 + 128], qc_hi,
            start=True, stop=True, skip_group_check=True,
        )

    emit_squad(0)
    emit_squad(1)
    ot2 = rs = None
    et_prev = None
    for gg in range(NGG):
        qi, g = gg // NG, gg % NG
        if g == 0:
            ot2 = opp.tile([128, QTW], F32, tag="ot2")
            rs = rsp.tile([97, QTW], F32, tag="rs")
        sp = sp_tiles.pop(gg)
        et = ep.tile([128, QTW * 2], BF16, tag="et")
        nc.scalar.activation(
            et[:], sp[:], mybir.ActivationFunctionType.Exp, scale=SCALE
        )
        for t in range(4):
            par = t // 2
            nc.tensor.matmul(
                ot2[32 * t : 32 * (t + 1), :],
                vq4[:, g, par, 32 * (t % 2) : 32 * (t % 2 + 1)],
                et[:, par * QTW : (par + 1) * QTW],
                start=(g == 0), stop=(g == NG - 1), skip_group_check=True,
                tile_position=(0, 32 * t),
            )
        if g % 2 == 1:
            for t, (esrc, half) in enumerate(
                [(et_prev, 0), (et_prev, 1), (et, 0), (et, 1)]
            ):
                nc.tensor.matmul(
                    rs[32 * t : 32 * t + 1, :],
                    ones[:],
                    esrc[:, half * QTW : (half + 1) * QTW],
                    start=(g == 1), stop=(g == NG - 1),
                    skip_group_check=True, tile_position=(0, 32 * t),
                )
        et_prev = et
        if gg + 2 < NGG:
            emit_squad(gg + 2)

        if g == NG - 1:
            # ---- Phase C for q-tile qi ----
            osb = fp.tile([128, QTW], BF16, tag="osb")
            nc.vector.tensor_copy(osb[:], ot2[:])
            rsb = fp.tile([97, QTW], BF16, tag="rsb")
            nc.vector.tensor_copy(rsb[:], rs[:])
            ctp = tpp.tile([128, 4 * 64], F32, tag="small")
            rcol = rsp.tile([128, 4], F32, tag="rs")
            for j in range(QTW // 128):
                js = slice(j * 128, (j + 1) * 128)
                nc.tensor.matmul(
                    ctp[:, j * 64 : (j + 1) * 64], osb[:, js], ident2[:],
                    start=True, stop=True, skip_group_check=True,
                )
                nc.tensor.matmul(
                    rcol[:, j : j + 1], rsb[:, js], wsel[:],
                    start=True, stop=True, skip_group_check=True,
                )
            rinv = fp.tile([128, 4], F32, tag="rinv")
            nc.vector.reciprocal(rinv[:], rcol[:])
            ob = fp.tile([128, 4 * 64], F32, tag="ob")
            for j in range(QTW // 128):
                nc.vector.tensor_scalar_mul(
                    ob[:, j * 64 : (j + 1) * 64],
                    ctp[:, j * 64 : (j + 1) * 64], rinv[:, j : j + 1]
                )
            nc.sync.dma_start(
                out=Od[b, qi * QTW : (qi + 1) * QTW].rearrange(
                    "(j p) d -> p j d", p=128
                ),
                in_=ob[:].rearrange("p (j d) -> p j d", d=64),
            )
            if qi == 0 and after_qt0 is not None:
                after_qt0()


def build_body(nc, tc, Qd, Kd, Vd, Od):
    with (
        tc.tile_pool(name="const", bufs=1) as constp,
        tc.tile_pool(name="qk", bufs=2) as qkp,
        tc.tile_pool(name="v", bufs=2) as vp,
        tc.tile_pool(name="nat", bufs=2) as natp,
        tc.tile_pool(name="dram", bufs=2, space="DRAM") as drp,
        tc.tile_pool(name="sp", bufs=2, space="PSUM") as spp,
        tc.tile_pool(name="ot", bufs=2, space="PSUM") as opp,
        tc.tile_pool(name="rs", bufs=1, space="PSUM") as rsp,
        tc.tile_pool(name="small", bufs=1, space="PSUM") as tpp,
        tc.tile_pool(name="et", bufs=4) as ep,
        tc.tile_pool(name="fin", bufs=3) as fp,
    ):
        pools = {
            "qk": qkp, "v": vp, "nat": natp, "dram": drp, "sp": spp,
            "ot": opp, "rs": rsp, "small": tpp, "et": ep, "fin": fp,
        }
        ident2 = constp.tile([128, 64], BF16)
        nc.gpsimd.memset(ident2[:], 0.0)
        for half in range(2):
            nc.gpsimd.affine_select(
                out=ident2[64 * half : 64 * (half + 1), :],
                in_=ident2[64 * half : 64 * (half + 1), :],
                compare_op=mybir.AluOpType.not_equal, fill=1.0, base=0,
                pattern=[[-1, 64]], channel_multiplier=1,
            )
        identB = constp.tile([128, 128], BF16)
        make_identity(nc, identB[:])
        ones = constp.tile([128, 1], BF16)
        nc.gpsimd.memset(ones[:], 1.0)
        wsel = constp.tile([97, 1], BF16)
        nc.gpsimd.memset(wsel[:], 0.0)
        for t in range(4):
            nc.gpsimd.memset(wsel[32 * t : 32 * t + 1, :], 1.0)

        handles = [None] * BS
        handles[0] = emit_phase_a(nc, pools, Qd, Kd, Vd, 0, identB)
        for b in range(BS):

            def prefetch(b=b):
                if b + 1 < BS:
                    handles[b + 1] = emit_phase_a(
                        nc, pools, Qd, Kd, Vd, b + 1, identB
                    )

            qt2, ktp, vq4 = handles[b]
            emit_phase_b(
                nc, pools, Od, b, qt2, ktp, vq4, ones, wsel, ident2, prefetch
            )


_nc_cache = None


def build_nc():
    global _nc_cache
    if _nc_cache is not None:
        return _nc_cache
    nc = bacc.Bacc(None, target_bir_lowering=False)
    Qd = nc.declare_dram_parameter("Q", [BS, S, D], F32, isOutput=False)
    Kd = nc.declare_dram_parameter("K", [BS, S, D], F32, isOutput=False)
    Vd = nc.declare_dram_parameter("V", [BS, S, D], F32, isOutput=False)
    Od = nc.declare_dram_parameter("out", [BS, S, D], F32, isOutput=True)
    with TileContext(nc) as tc:
        build_body(nc, tc, Qd, Kd, Vd, Od)
    nc.finalize()
    _nc_cache = nc
    return nc


def kernel(Q, K, V):
    Q = np.asarray(Q, dtype=np.float32)
    K = np.asarray(K, dtype=np.float32)
    V = np.asarray(V, dtype=np.float32)
    nc = build_nc()
    in_maps = [
        {
            "Q": np.ascontiguousarray(Q[i * BS : (i + 1) * BS]),
            "K": np.ascontiguousarray(K[i * BS : (i + 1) * BS]),
            "V": np.ascontiguousarray(V[i * BS : (i + 1) * BS]),
        }
        for i in range(N_CORES)
    ]
    res = run_bass_kernel_spmd(nc, in_maps, core_ids=list(range(N_CORES)))
    return np.concatenate([res.results[i]["out"] for i in range(N_CORES)], axis=0)
